# revision 21
# baseline (speedup 1.0000x reference)
"""Expert-parallel MoE (top-1 routing) Bass kernel for Trainium2, 8 cores.

Strategy (core c = expert c, SPMD — one program, per-core weight slices):
  1. Router logits via exact fp32 PE matmul (logitsT [E, N], experts on
     partitions, tokens on free dim).
  2. PE-transpose to token-major [128, 16, E]; per-token argmax via
     reduce_max + is_equal + min(eq*(iota-8))+8  (first-max tie-break,
     matching jax.lax.top_k).
  3. mask = (sel == c); compaction positions via matmul-based exclusive
     cumsum (strict-upper-triangular one-matrices); unselected -> -1.
  4. One-hot dispatch matrix O [N, C] bf16 built by f32 iota is_equal;
     gather matmul xcT[d, j] = sum_n x[n, d] * O[n, j]  (bf16), paced so
     the first d-chunk pair chases the O build.
  5. SwiGLU FFN on the C compacted tokens (bf16 matmuls, fp32 PSUM):
     per 128-row H chunk: a = W1 xc, b = W2 xc, h = silu(a)*b; the
     combine ycT[d, c] += W3 h runs as two D-half passes (4 PSUM
     accumulators each) over a resident W3, giving zero M-padding waste.
  6. Aux loss (z-loss + load-balance CV^2) fully on device from the
     token-major logits (|logit| <~ 6 for N(0,1)-scaled inputs, so exp
     without max-subtraction is safe in fp32).
  7. Host unshard: out[token_ids(c)] = ycT_c.T[:count_c]; aux from core 0.
     Tokens beyond the C=320 capacity (never observed; max count 298)
     fall back to an exact host-side computation.

Inputs (full, unsharded): x [2,1024,1024] f32, Wr [8,1024] f32,
W1 [8,3584,1024] f32, W2 [8,3584,1024] f32, W3 [8,1024,3584] f32.
Returns (out [2,1024,1024] f32, aux_loss f32 scalar) like the reference.
"""

import os
# The axon client in this container has no NTFF profiling hook; a stray
# BASS_TRACE=1 in the environment would crash run_bass_kernel_spmd.
os.environ["BASS_NEVER_TRACE"] = "1"

import numpy as np
import ml_dtypes

import concourse.bacc as bacc
import concourse.tile as tile
from concourse.tile import add_dep_helper
import concourse.mybir as mybir
from concourse.bass_utils import run_bass_kernel_spmd

P = 128
D = 1024
HID = 3584
E = 8
N = 2048
C = 320          # per-expert token capacity (max observed count ~298)
KD = D // P      # 8 k-tiles over D
KH = HID // P    # 28 k-tiles over H
NT = N // P      # 16 token tiles
F32 = mybir.dt.float32
F16 = mybir.dt.float16
BF16 = mybir.dt.bfloat16
AF = mybir.ActivationFunctionType
OP = mybir.AluOpType

_CACHE = {}


def _build():
    nc = bacc.Bacc(None, target_bir_lowering=False)
    names = {}
    with tile.TileContext(nc) as tc:
        with tc.tile_pool(name="dram", bufs=1, space="DRAM") as dram:
            # ---------------- I/O ----------------
            d_xT = dram.tile([KD, P, N], F32, kind="ExternalInput")
            d_xtok = dram.tile([P, NT, D], BF16, kind="ExternalInput")
            d_wrT = dram.tile([P, KD, E], F32, kind="ExternalInput")
            d_w1 = dram.tile([KH, P, KD, P], BF16, kind="ExternalInput")
            d_w2 = dram.tile([KH, P, KD, P], BF16, kind="ExternalInput")
            d_w3 = dram.tile([KH, P, D], BF16, kind="ExternalInput")
            d_ecol = dram.tile([P, 1], F32, kind="ExternalInput")
            # constants
            d_tri128 = dram.tile([P, P], F32, kind="ExternalInput")
            d_tri16 = dram.tile([16, 16], F32, kind="ExternalInput")
            d_ones128 = dram.tile([P, 1], F32, kind="ExternalInput")
            d_ones1x = dram.tile([1, P], F32, kind="ExternalInput")
            d_iotaC = dram.tile([P, C], F32, kind="ExternalInput")
            d_iotam8 = dram.tile([P, NT, E], F32, kind="ExternalInput")
            d_ident = dram.tile([P, P], F32, kind="ExternalInput")
            # outputs
            d_yct = dram.tile([KD, P, C], F32, kind="ExternalOutput")
            d_sel = dram.tile([P, NT], F32, kind="ExternalOutput")
            d_aux = dram.tile([1, 1], F32, kind="ExternalOutput")
            names = dict(
                xT=d_xT.name, xtok=d_xtok.name, wrT=d_wrT.name,
                w1=d_w1.name, w2=d_w2.name, w3=d_w3.name, ecol=d_ecol.name,
                tri128=d_tri128.name, tri16=d_tri16.name,
                ones128=d_ones128.name, ones1x=d_ones1x.name,
                iotaC=d_iotaC.name, iotam8=d_iotam8.name, ident=d_ident.name,
                yc=d_yct.name, sel=d_sel.name, aux=d_aux.name,
            )

            with (
                tc.tile_pool(name="const", bufs=1) as constp,
                tc.tile_pool(name="persist", bufs=1) as persist,
                tc.tile_pool(name="w12", bufs=3) as w12,
                tc.tile_pool(name="w3p", bufs=3) as w3p,
                tc.tile_pool(name="silp", bufs=2) as silp,
                tc.tile_pool(name="ycp", bufs=1) as ycp,
            ):
                # persistent tensors (x_tok DMA deferred until gather phase)
                x_tok = persist.tile([P, NT, D], BF16, tag="x_tok")
                wr_all = persist.tile([P, KD, E], F32, tag="wr_all")
                nc.sync.dma_start(wr_all[:], d_wrT[:])
                logits_sb = persist.tile([E, N], F32, tag="logits_sb")
                lt_all = persist.tile([P, NT, E], F32, tag="lt_all")
                O = persist.tile([P, NT, C], BF16, tag="O")
                xcT = persist.tile([P, KD, C], BF16, tag="xcT")
                hT = persist.tile([P, KH, C], BF16, tag="hT")
                w3all = persist.tile([P, KH, D], BF16, tag="w3all")
                sel = persist.tile([P, NT], F32, tag="sel")
                mask = persist.tile([P, NT], F32, tag="mask")

                # ---------------- Phase R: router (fp32) ----------------
                with (
                    tc.tile_pool(name="xr", bufs=3) as xrp,
                    tc.tile_pool(name="psR", bufs=1, space="PSUM") as psR,
                ):
                    ps_l = [
                        psR.tile([E, 512], F32, tag=f"l{j}", name=f"ps_l{j}")
                        for j in range(4)
                    ]
                    xts = []
                    last_xt_dma = None
                    for kt in range(KD):
                        xt = xrp.tile([P, N], F32, tag="xr", name=f"xt{kt}")
                        for q in range(4):
                            qsl = slice(q * 512, (q + 1) * 512)
                            last_xt_dma = nc.sync.dma_start(
                                xt[:, qsl], d_xT[kt][:, qsl])
                        xts.append(xt)
                    # constants (issued after the router stream in program order)
                    tri128 = constp.tile([P, P], F32, tag="tri128")
                    nc.sync.dma_start(tri128[:], d_tri128[:])
                    tri16 = constp.tile([16, 16], F32, tag="tri16")
                    nc.sync.dma_start(tri16[:], d_tri16[:])
                    ones128 = constp.tile([P, 1], F32, tag="ones128")
                    nc.sync.dma_start(ones128[:], d_ones128[:])
                    ones1x = constp.tile([1, P], F32, tag="ones1x")
                    nc.sync.dma_start(ones1x[:], d_ones1x[:])
                    iotaC = constp.tile([P, C], F32, tag="iotaC")
                    nc.sync.dma_start(iotaC[:], d_iotaC[:])
                    iotam8 = constp.tile([P, NT, E], F32, tag="iotam8")
                    nc.sync.dma_start(iotam8[:], d_iotam8[:])
                    ident = constp.tile([P, P], F32, tag="ident")
                    nc.sync.dma_start(ident[:], d_ident[:])
                    ecol = constp.tile([P, 1], F32, tag="ecol")
                    nc.sync.dma_start(ecol[:], d_ecol[:])
                    for kt in range(KD):
                        for j in range(4):
                            sl = slice(j * 512, (j + 1) * 512)
                            nc.tensor.matmul(
                                ps_l[j][:], wr_all[:, kt, :], xts[kt][:, sl],
                                start=(kt == 0), stop=(kt == KD - 1),
                            )
                    for j in range(4):
                        sl = slice(j * 512, (j + 1) * 512)
                        if j % 2 == 0:
                            nc.scalar.copy(logits_sb[:, sl], ps_l[j][:])
                        else:
                            nc.vector.tensor_copy(logits_sb[:, sl], ps_l[j][:])
                # x_tok transfer queued right after the router stream
                prev_dma = last_xt_dma
                for q in range(4):
                    qsl = slice(q * 4, (q + 1) * 4)
                    xtok_dma = nc.sync.dma_start(x_tok[:, qsl, :], d_xtok[:, qsl, :])
                    add_dep_helper(xtok_dma.ins, prev_dma.ins, sync=False,
                                   reason="x_tok after router stream, in order")
                    prev_dma = xtok_dma

                # ------- Phase T/X/G: argmax, pos, aux, one-hot, gather -------
                with (
                    tc.tile_pool(name="psT", bufs=2, space="PSUM") as psT,
                    tc.tile_pool(name="psD", bufs=2, space="PSUM") as psD,
                    tc.tile_pool(name="psG", bufs=2, space="PSUM") as psG,
                    tc.tile_pool(name="tmpT", bufs=1) as tmp,
                    tc.tile_pool(name="tmpX", bufs=1) as tx,
                ):
                    m_max = tmp.tile([P, NT], F32, tag="m_max")
                    eq = tmp.tile([P, NT, E], F32, tag="eq")
                    for t in range(NT):
                        pst = psT.tile([P, E], F32, tag="pst")
                        nc.tensor.transpose(
                            pst[:], logits_sb[:, t * P:(t + 1) * P],
                            ident[:E, :E]
                        )
                        if t % 2 == 0:
                            nc.vector.tensor_copy(lt_all[:, t, :], pst[:])
                        else:
                            nc.scalar.copy(lt_all[:, t, :], pst[:])
                        nc.vector.tensor_reduce(
                            m_max[:, t:t + 1], lt_all[:, t, :],
                            mybir.AxisListType.X, OP.max
                        )
                        nc.vector.tensor_scalar(
                            eq[:, t, :], lt_all[:, t, :],
                            m_max[:, t:t + 1], None, OP.is_equal,
                        )
                    nc.vector.tensor_mul(eq[:], eq[:], iotam8[:])
                    selm = tmp.tile([P, NT], F32, tag="selm")
                    nc.vector.tensor_reduce(
                        selm[:], eq[:], mybir.AxisListType.X, OP.min
                    )
                    nc.vector.tensor_scalar_add(sel[:], selm[:], 8.0)
                    nc.vector.tensor_scalar(
                        mask[:], sel[:], ecol[:, 0:1], None, OP.is_equal
                    )
                    # exclusive cumsum of mask in token order (n = t*128 + p)
                    ps_cs = psD.tile([16, 1], F32, tag="small")
                    nc.tensor.matmul(ps_cs[:], mask[:], ones128[:])
                    colsum = tmp.tile([16, 1], F32, tag="colsum")
                    nc.vector.tensor_copy(colsum[:], ps_cs[:])
                    ps_co = psD.tile([1, NT], F32, tag="small")
                    nc.tensor.matmul(ps_co[:], colsum[:], tri16[:])
                    coloff = tmp.tile([1, NT], F32, tag="coloff")
                    nc.vector.tensor_copy(coloff[:], ps_co[:])
                    ps_pos = psD.tile([P, NT], F32, tag="small")
                    nc.tensor.matmul(ps_pos[:], tri128[:], mask[:],
                                     start=True, stop=False)
                    nc.tensor.matmul(ps_pos[:], ones1x[:], coloff[:],
                                     start=False, stop=True)
                    mm1 = tmp.tile([P, NT], F32, tag="mm1")
                    nc.vector.tensor_scalar_add(mm1[:], mask[:], -1.0)
                    pos = tmp.tile([P, NT], F32, tag="pos")
                    # pos_m = pos*mask + (mask-1):  -1 where unselected
                    nc.vector.tensor_mul(pos[:], ps_pos[:], mask[:])
                    nc.vector.tensor_add(pos[:], pos[:], mm1[:])
                    for t in range(NT):
                        nc.vector.tensor_scalar(
                            O[:, t, :], iotaC[:], pos[:, t:t + 1], None,
                            OP.is_equal,
                        )
                    # gather: dc-pairs, t-inner so the first pair chases O
                    for pr in range(KD // 2):
                        pg0 = psG.tile([P, C], F32, tag="g0", name=f"pg0_{pr}")
                        pg1 = psG.tile([P, C], F32, tag="g1", name=f"pg1_{pr}")
                        d0 = 2 * pr
                        for t in range(NT):
                            nc.tensor.matmul(
                                pg0[:], x_tok[:, t, d0 * P:(d0 + 1) * P],
                                O[:, t, :],
                                start=(t == 0), stop=(t == NT - 1),
                            )
                            nc.tensor.matmul(
                                pg1[:], x_tok[:, t, (d0 + 1) * P:(d0 + 2) * P],
                                O[:, t, :],
                                start=(t == 0), stop=(t == NT - 1),
                            )
                        nc.vector.tensor_copy(xcT[:, d0, :], pg0[:])
                        nc.vector.tensor_copy(xcT[:, d0 + 1, :], pg1[:])

                    # ---------------- aux loss (overlaps gather) ----------------
                    exp_all = tx.tile([P, NT, E], F32, tag="exp_all")
                    nc.scalar.activation(exp_all[:], lt_all[:], AF.Exp)
                    sumexp = tx.tile([P, NT], F32, tag="sumexp")
                    nc.vector.tensor_reduce(
                        sumexp[:], exp_all[:], mybir.AxisListType.X, OP.add
                    )
                    lse = tx.tile([P, NT], F32, tag="lse")
                    lse_sum = tx.tile([P, 1], F32, tag="lse_sum")
                    nc.scalar.activation(
                        lse[:], sumexp[:], AF.Ln, accum_out=lse_sum[:]
                    )
                    ps_z = psD.tile([1, 1], F32, tag="small")
                    nc.tensor.matmul(ps_z[:], lse_sum[:], ones128[:])
                    recip = tx.tile([P, NT], F32, tag="recip")
                    nc.vector.reciprocal(recip[:], sumexp[:])
                    probs = tx.tile([P, NT, E], F32, tag="probs")
                    for t in range(NT):
                        nc.vector.tensor_scalar(
                            probs[:, t, :], exp_all[:, t, :],
                            recip[:, t:t + 1], None, OP.mult,
                        )
                    load = tx.tile([P, NT // 2, E], F32, tag="load")
                    nc.vector.tensor_add(
                        load[:], probs[:, 0:NT // 2, :], probs[:, NT // 2:NT, :]
                    )
                    s1 = tx.tile([P, 1], F32, tag="s1")
                    nc.vector.tensor_reduce(
                        s1[:], load[:], mybir.AxisListType.XY, OP.add
                    )
                    sq = tx.tile([P, NT // 2, E], F32, tag="sq")
                    s2 = tx.tile([P, 1], F32, tag="s2")
                    nc.scalar.activation(sq[:], load[:], AF.Square,
                                         accum_out=s2[:])
                    stats = tx.tile([P, 2], F32, tag="stats")
                    nc.vector.tensor_copy(stats[:, 0:1], s1[:])
                    nc.vector.tensor_copy(stats[:, 1:2], s2[:])
                    ps_s = psD.tile([1, 2], F32, tag="small")
                    nc.tensor.matmul(ps_s[:], ones128[:], stats[:])
                    st = tx.tile([1, 2], F32, tag="st")
                    nc.vector.tensor_copy(st[:], ps_s[:])
                    zt = tx.tile([1, 1], F32, tag="zt")
                    nc.vector.tensor_copy(zt[:], ps_z[:])
                    # scalar math
                    NL = float(1024 * E)
                    m2 = tx.tile([1, 1], F32, tag="m2")
                    nc.vector.tensor_mul(m2[:], st[:, 0:1], st[:, 0:1])
                    t3 = tx.tile([1, 1], F32, tag="t3")
                    nc.vector.tensor_scalar_mul(t3[:], m2[:], 1.0 / NL)
                    v1 = tx.tile([1, 1], F32, tag="v1")
                    nc.vector.tensor_sub(v1[:], st[:, 1:2], t3[:])
                    var = tx.tile([1, 1], F32, tag="var")
                    nc.vector.tensor_scalar_mul(var[:], v1[:], 1.0 / (NL - 1.0))
                    mean2 = tx.tile([1, 1], F32, tag="mean2")
                    nc.vector.tensor_scalar_mul(mean2[:], m2[:], 1.0 / (NL * NL))
                    rm = tx.tile([1, 1], F32, tag="rm")
                    nc.vector.reciprocal(rm[:], mean2[:])
                    ratio = tx.tile([1, 1], F32, tag="ratio")
                    nc.vector.tensor_mul(ratio[:], var[:], rm[:])
                    mz = tx.tile([1, 1], F32, tag="mz")
                    nc.vector.tensor_scalar_mul(mz[:], zt[:], 1.0 / float(N))
                    z2 = tx.tile([1, 1], F32, tag="z2")
                    nc.vector.tensor_mul(z2[:], mz[:], mz[:])
                    t4 = tx.tile([1, 1], F32, tag="t4")
                    nc.vector.tensor_add(t4[:], z2[:], ratio[:])
                    auxv = tx.tile([1, 1], F32, tag="auxv")
                    nc.vector.tensor_scalar_mul(auxv[:], t4[:], 1.0e-3)
                    nc.scalar.dma_start(d_aux[:], auxv[:])
                nc.scalar.dma_start(d_sel[:], sel[:])

                # ---------------- Phase F: FFN ----------------
                with (
                    tc.tile_pool(name="psF", bufs=1, space="PSUM") as psF,
                ):
                    ps_yct = [
                        psF.tile([P, C], F32, tag=f"yt{i}", name=f"ps_yct{i}")
                        for i in range(4)
                    ]
                    for hc in range(KH):
                        w1t = w12.tile([P, KD, P], BF16, tag="w1")
                        nc.sync.dma_start(w1t[:], d_w1[hc][:])
                        w2t = w12.tile([P, KD, P], BF16, tag="w2")
                        nc.sync.dma_start(w2t[:], d_w2[hc][:])
                        nc.sync.dma_start(w3all[:, hc, :], d_w3[hc][:])
                        ps_a = psF.tile([P, C], F32, tag="a")
                        ps_b = psF.tile([P, C], F32, tag="b")
                        for kt in range(KD):
                            nc.tensor.matmul(
                                ps_a[:], w1t[:, kt, :], xcT[:, kt, :],
                                start=(kt == 0), stop=(kt == KD - 1),
                            )
                        for kt in range(KD):
                            nc.tensor.matmul(
                                ps_b[:], w2t[:, kt, :], xcT[:, kt, :],
                                start=(kt == 0), stop=(kt == KD - 1),
                            )
                        sil = silp.tile([P, C], F32, tag="sil")
                        nc.scalar.activation(sil[:], ps_a[:], AF.Silu)
                        nc.vector.tensor_mul(hT[:, hc, :], sil[:], ps_b[:])
                        # FFN3 pass A: d-chunks 0..3, accumulated across hc
                        for i in range(4):
                            nc.tensor.matmul(
                                ps_yct[i][:],
                                w3all[:, hc, i * P:(i + 1) * P],
                                hT[:, hc, :],
                                start=(hc == 0), stop=(hc == KH - 1),
                            )
                    ycte = ycp.tile([P, 4, C], F32, tag="ycteA")
                    for i in range(4):
                        if i % 2 == 0:
                            nc.scalar.copy(ycte[:, i, :], ps_yct[i][:])
                        else:
                            nc.vector.tensor_copy(ycte[:, i, :], ps_yct[i][:])
                        nc.sync.dma_start(d_yct[i][:], ycte[:, i, :])
                    # FFN3 pass B: d-chunks 4..7 over resident w3/hT
                    ps_yct2 = [
                        psF.tile([P, C], F32, tag=f"yt{i}", name=f"ps_yct2_{i}")
                        for i in range(4)
                    ]
                    for hc in range(KH):
                        for i in range(4):
                            nc.tensor.matmul(
                                ps_yct2[i][:],
                                w3all[:, hc, (4 + i) * P:(5 + i) * P],
                                hT[:, hc, :],
                                start=(hc == 0), stop=(hc == KH - 1),
                            )
                    ycte2 = ycp.tile([P, 4, C], F32, tag="ycteB")
                    for i in range(4):
                        if i % 2 == 0:
                            nc.scalar.copy(ycte2[:, i, :], ps_yct2[i][:])
                        else:
                            nc.vector.tensor_copy(ycte2[:, i, :], ps_yct2[i][:])
                        nc.sync.dma_start(d_yct[4 + i][:], ycte2[:, i, :])
    nc.compile()
    return nc, names


def _prep_host(x, Wr, W1, W2, W3):
    flat = np.ascontiguousarray(x.reshape(N, D), dtype=np.float32)
    xT = np.ascontiguousarray(flat.T).reshape(KD, P, N)
    x_tok = np.ascontiguousarray(
        flat.reshape(NT, P, D).transpose(1, 0, 2)
    ).astype(ml_dtypes.bfloat16)
    # wrT[p, kt, e] = Wr[e, kt*128+p]
    wrT = np.ascontiguousarray(
        Wr.T.reshape(KD, P, E).transpose(1, 0, 2), dtype=np.float32
    )
    bf = ml_dtypes.bfloat16
    # w1c[e, hc, p, kt, m] = W1[e, hc*128+m, kt*128+p]
    w1c = W1.reshape(E, KH, P, KD, P).transpose(0, 1, 4, 3, 2).astype(bf)
    w2c = W2.reshape(E, KH, P, KD, P).transpose(0, 1, 4, 3, 2).astype(bf)
    w3s = np.ascontiguousarray(
        W3.transpose(0, 2, 1).reshape(E, KH, P, D)
    ).astype(bf)
    consts = dict(
        tri128=np.triu(np.ones((P, P), np.float32), 1),
        tri16=np.triu(np.ones((16, 16), np.float32), 1),
        ones128=np.ones((P, 1), np.float32),
        ones1x=np.ones((1, P), np.float32),
        iotaC=np.broadcast_to(
            np.arange(C, dtype=np.float32), (P, C)
        ).copy(),
        iotam8=np.broadcast_to(
            (np.arange(E) - 8.0).astype(np.float32), (P, NT, E)
        ).copy(),
        ident=np.eye(P, dtype=np.float32),
    )
    return flat, xT, x_tok, wrT, w1c, w2c, w3s, consts


def kernel(x, Wr, W1, W2, W3):
    x = np.asarray(x, dtype=np.float32)
    Wr = np.asarray(Wr, dtype=np.float32)
    W1 = np.asarray(W1, dtype=np.float32)
    W2 = np.asarray(W2, dtype=np.float32)
    W3 = np.asarray(W3, dtype=np.float32)

    if "nc" not in _CACHE:
        _CACHE["nc"] = _build()
    nc, nm = _CACHE["nc"]

    flat, xT, x_tok, wrT, w1c, w2c, w3s, consts = _prep_host(x, Wr, W1, W2, W3)

    in_maps = []
    for c in range(E):
        ecol = np.full((P, 1), float(c), np.float32)
        in_maps.append({
            nm["xT"]: xT, nm["xtok"]: x_tok, nm["wrT"]: wrT,
            nm["w1"]: w1c[c], nm["w2"]: w2c[c], nm["w3"]: w3s[c],
            nm["ecol"]: ecol,
            nm["tri128"]: consts["tri128"], nm["tri16"]: consts["tri16"],
            nm["ones128"]: consts["ones128"], nm["ones1x"]: consts["ones1x"],
            nm["iotaC"]: consts["iotaC"], nm["iotam8"]: consts["iotam8"],
            nm["ident"]: consts["ident"],
        })
    res = run_bass_kernel_spmd(nc, in_maps, list(range(E)))
    _CACHE["last_res"] = res

    sel_arr = res.results[0][nm["sel"]]          # [128, 16], token n = t*128+p
    sel_flat = sel_arr.flatten(order="F").astype(np.int64)  # [2048]
    _CACHE["sel_flat"] = sel_flat
    out = np.zeros((N, D), dtype=np.float32)
    flat = x.reshape(N, D)
    for c in range(E):
        idx = np.where(sel_flat == c)[0]
        ycT = res.results[c][nm["yc"]].reshape(D, C)  # [KD*P, C]
        take = min(len(idx), C)
        out[idx[:take]] = ycT[:, :take].T
        if len(idx) > C:
            # capacity overflow: exact host fallback for the tail tokens
            xo = flat[idx[C:]]
            a = xo @ W1[c].T
            h = (a / (1.0 + np.exp(-a))) * (xo @ W2[c].T)
            out[idx[C:]] = h @ W3[c].T
    aux = np.float32(res.results[0][nm["aux"]][0, 0])
    return out.reshape(2, 1024, D), aux


if __name__ == "__main__":
    import json
    rng = np.random.default_rng(0)
    x = rng.standard_normal((2, 1024, D), dtype=np.float32)
    Wr = (rng.standard_normal((E, D), dtype=np.float32) / np.sqrt(D))
    W1 = (rng.standard_normal((E, HID, D), dtype=np.float32) / np.sqrt(D))
    W2 = (rng.standard_normal((E, HID, D), dtype=np.float32) / np.sqrt(D))
    W3 = (rng.standard_normal((E, D, HID), dtype=np.float32) / np.sqrt(HID))
    out, aux = kernel(x, Wr, W1, W2, W3)
    print("out", out.shape, out.dtype, "aux", aux)


# revision 24
# speedup vs baseline: 1.0066x; 1.0066x over previous
"""Expert-parallel MoE (top-1 routing) Bass kernel for Trainium2, 8 cores.

Strategy (core c = expert c, SPMD — one program, per-core weight slices):
  1. Router logits via exact fp32 PE matmul (logitsT [E, N], experts on
     partitions, tokens on free dim).
  2. PE-transpose to token-major [128, 16, E]; per-token argmax via
     reduce_max + is_equal + min(eq*(iota-8))+8  (first-max tie-break,
     matching jax.lax.top_k).
  3. mask = (sel == c); compaction positions via matmul-based exclusive
     cumsum (strict-upper-triangular one-matrices); unselected -> -1.
  4. One-hot dispatch matrix O [N, C] bf16 built by f32 iota is_equal;
     gather matmul xcT[d, j] = sum_n x[n, d] * O[n, j]  (bf16), paced so
     the first d-chunk pair chases the O build.
  5. SwiGLU FFN on the C compacted tokens (bf16 matmuls, fp32 PSUM):
     per 128-row H chunk: a = W1 xc, b = W2 xc, h = silu(a)*b; the
     combine ycT[d, c] += W3 h runs as two passes (6 then 2 PSUM
     accumulators) over a resident W3, giving zero M-padding waste.
  6. Aux loss (z-loss + load-balance CV^2) fully on device from the
     token-major logits (|logit| <~ 6 for N(0,1)-scaled inputs, so exp
     without max-subtraction is safe in fp32).
  7. Host unshard: out[token_ids(c)] = ycT_c.T[:count_c]; aux from core 0.
     Tokens beyond the C=320 capacity (never observed; max count 298)
     fall back to an exact host-side computation.

Inputs (full, unsharded): x [2,1024,1024] f32, Wr [8,1024] f32,
W1 [8,3584,1024] f32, W2 [8,3584,1024] f32, W3 [8,1024,3584] f32.
Returns (out [2,1024,1024] f32, aux_loss f32 scalar) like the reference.
"""

import os
# The axon client in this container has no NTFF profiling hook; a stray
# BASS_TRACE=1 in the environment would crash run_bass_kernel_spmd.
os.environ["BASS_NEVER_TRACE"] = "1"

import numpy as np
import ml_dtypes

import concourse.bacc as bacc
import concourse.tile as tile
from concourse.tile import add_dep_helper
import concourse.mybir as mybir
from concourse.bass_utils import run_bass_kernel_spmd

P = 128
D = 1024
HID = 3584
E = 8
N = 2048
C = 320          # per-expert token capacity (max observed count ~298)
KD = D // P      # 8 k-tiles over D
KH = HID // P    # 28 k-tiles over H
NT = N // P      # 16 token tiles
F32 = mybir.dt.float32
F16 = mybir.dt.float16
BF16 = mybir.dt.bfloat16
AF = mybir.ActivationFunctionType
OP = mybir.AluOpType

_CACHE = {}


def _build():
    nc = bacc.Bacc(None, target_bir_lowering=False)
    names = {}
    with tile.TileContext(nc) as tc:
        with tc.tile_pool(name="dram", bufs=1, space="DRAM") as dram:
            # ---------------- I/O ----------------
            d_xT = dram.tile([KD, P, N], F32, kind="ExternalInput")
            d_xtok = dram.tile([P, NT, D], BF16, kind="ExternalInput")
            d_wrT = dram.tile([P, KD, E], F32, kind="ExternalInput")
            d_w1 = dram.tile([KH, P, KD, P], BF16, kind="ExternalInput")
            d_w2 = dram.tile([KH, P, KD, P], BF16, kind="ExternalInput")
            d_w3 = dram.tile([KH, P, D], BF16, kind="ExternalInput")
            d_ecol = dram.tile([P, 1], F32, kind="ExternalInput")
            # constants
            d_tri128 = dram.tile([P, P], F32, kind="ExternalInput")
            d_tri16 = dram.tile([16, 16], F32, kind="ExternalInput")
            d_ones128 = dram.tile([P, 1], F32, kind="ExternalInput")
            d_ones1x = dram.tile([1, P], F32, kind="ExternalInput")
            d_iotaC = dram.tile([P, C], F32, kind="ExternalInput")
            d_iotam8 = dram.tile([P, NT, E], F32, kind="ExternalInput")
            d_ident = dram.tile([P, P], F32, kind="ExternalInput")
            # outputs
            d_yct = dram.tile([KD, P, C], F32, kind="ExternalOutput")
            d_sel = dram.tile([P, NT], F32, kind="ExternalOutput")
            d_aux = dram.tile([1, 1], F32, kind="ExternalOutput")
            names = dict(
                xT=d_xT.name, xtok=d_xtok.name, wrT=d_wrT.name,
                w1=d_w1.name, w2=d_w2.name, w3=d_w3.name, ecol=d_ecol.name,
                tri128=d_tri128.name, tri16=d_tri16.name,
                ones128=d_ones128.name, ones1x=d_ones1x.name,
                iotaC=d_iotaC.name, iotam8=d_iotam8.name, ident=d_ident.name,
                yc=d_yct.name, sel=d_sel.name, aux=d_aux.name,
            )

            with (
                tc.tile_pool(name="const", bufs=1) as constp,
                tc.tile_pool(name="persist", bufs=1) as persist,
                tc.tile_pool(name="w12", bufs=3) as w12,
                tc.tile_pool(name="w3p", bufs=3) as w3p,
                tc.tile_pool(name="silp", bufs=2) as silp,
                tc.tile_pool(name="ycp", bufs=1) as ycp,
            ):
                # persistent tensors (x_tok DMA deferred until gather phase)
                x_tok = persist.tile([P, NT, D], BF16, tag="x_tok")
                wr_all = persist.tile([P, KD, E], F32, tag="wr_all")
                nc.sync.dma_start(wr_all[:], d_wrT[:])
                logits_sb = persist.tile([E, N], F32, tag="logits_sb")
                lt_all = persist.tile([P, NT, E], F32, tag="lt_all")
                O = persist.tile([P, NT, C], BF16, tag="O")
                xcT = persist.tile([P, KD, C], BF16, tag="xcT")
                hT = persist.tile([P, KH, C], BF16, tag="hT")
                w3all = persist.tile([P, KH, D], BF16, tag="w3all")
                sel = persist.tile([P, NT], F32, tag="sel")
                mask = persist.tile([P, NT], F32, tag="mask")

                # ---------------- Phase R: router (fp32) ----------------
                with (
                    tc.tile_pool(name="xr", bufs=3) as xrp,
                    tc.tile_pool(name="psR", bufs=1, space="PSUM") as psR,
                ):
                    ps_l = [
                        psR.tile([E, 512], F32, tag=f"l{j}", name=f"ps_l{j}")
                        for j in range(4)
                    ]
                    xts = []
                    last_xt_dma = None
                    for kt in range(KD):
                        xt = xrp.tile([P, N], F32, tag="xr", name=f"xt{kt}")
                        for q in range(4):
                            qsl = slice(q * 512, (q + 1) * 512)
                            last_xt_dma = nc.sync.dma_start(
                                xt[:, qsl], d_xT[kt][:, qsl])
                        xts.append(xt)
                    # constants (issued after the router stream in program order)
                    tri128 = constp.tile([P, P], F32, tag="tri128")
                    nc.sync.dma_start(tri128[:], d_tri128[:])
                    tri16 = constp.tile([16, 16], F32, tag="tri16")
                    nc.sync.dma_start(tri16[:], d_tri16[:])
                    ones128 = constp.tile([P, 1], F32, tag="ones128")
                    nc.sync.dma_start(ones128[:], d_ones128[:])
                    ones1x = constp.tile([1, P], F32, tag="ones1x")
                    nc.sync.dma_start(ones1x[:], d_ones1x[:])
                    iotaC = constp.tile([P, C], F32, tag="iotaC")
                    nc.sync.dma_start(iotaC[:], d_iotaC[:])
                    iotam8 = constp.tile([P, NT, E], F32, tag="iotam8")
                    nc.sync.dma_start(iotam8[:], d_iotam8[:])
                    ident = constp.tile([P, P], F32, tag="ident")
                    nc.sync.dma_start(ident[:], d_ident[:])
                    ecol = constp.tile([P, 1], F32, tag="ecol")
                    nc.sync.dma_start(ecol[:], d_ecol[:])
                    for kt in range(KD):
                        for j in range(4):
                            sl = slice(j * 512, (j + 1) * 512)
                            nc.tensor.matmul(
                                ps_l[j][:], wr_all[:, kt, :], xts[kt][:, sl],
                                start=(kt == 0), stop=(kt == KD - 1),
                            )
                    for j in range(4):
                        sl = slice(j * 512, (j + 1) * 512)
                        if j % 2 == 0:
                            nc.scalar.copy(logits_sb[:, sl], ps_l[j][:])
                        else:
                            nc.vector.tensor_copy(logits_sb[:, sl], ps_l[j][:])
                # x_tok transfer queued right after the router stream
                prev_dma = last_xt_dma
                for q in range(4):
                    qsl = slice(q * 4, (q + 1) * 4)
                    xtok_dma = nc.sync.dma_start(x_tok[:, qsl, :], d_xtok[:, qsl, :])
                    add_dep_helper(xtok_dma.ins, prev_dma.ins, sync=False,
                                   reason="x_tok after router stream, in order")
                    prev_dma = xtok_dma

                # ------- Phase T/X/G: argmax, pos, aux, one-hot, gather -------
                with (
                    tc.tile_pool(name="psT", bufs=2, space="PSUM") as psT,
                    tc.tile_pool(name="psD", bufs=2, space="PSUM") as psD,
                    tc.tile_pool(name="psG", bufs=2, space="PSUM") as psG,
                    tc.tile_pool(name="tmpT", bufs=1) as tmp,
                    tc.tile_pool(name="tmpX", bufs=1) as tx,
                ):
                    m_max = tmp.tile([P, NT], F32, tag="m_max")
                    eq = tmp.tile([P, NT, E], F32, tag="eq")
                    for t in range(NT):
                        pst = psT.tile([P, E], F32, tag="pst")
                        nc.tensor.transpose(
                            pst[:], logits_sb[:, t * P:(t + 1) * P],
                            ident[:E, :E]
                        )
                        if t % 2 == 0:
                            nc.vector.tensor_copy(lt_all[:, t, :], pst[:])
                        else:
                            nc.scalar.copy(lt_all[:, t, :], pst[:])
                        nc.vector.tensor_reduce(
                            m_max[:, t:t + 1], lt_all[:, t, :],
                            mybir.AxisListType.X, OP.max
                        )
                        nc.vector.tensor_scalar(
                            eq[:, t, :], lt_all[:, t, :],
                            m_max[:, t:t + 1], None, OP.is_equal,
                        )
                    nc.vector.tensor_mul(eq[:], eq[:], iotam8[:])
                    selm = tmp.tile([P, NT], F32, tag="selm")
                    nc.vector.tensor_reduce(
                        selm[:], eq[:], mybir.AxisListType.X, OP.min
                    )
                    nc.vector.tensor_scalar_add(sel[:], selm[:], 8.0)
                    nc.vector.tensor_scalar(
                        mask[:], sel[:], ecol[:, 0:1], None, OP.is_equal
                    )
                    # exclusive cumsum of mask in token order (n = t*128 + p)
                    ps_cs = psD.tile([16, 1], F32, tag="small")
                    nc.tensor.matmul(ps_cs[:], mask[:], ones128[:])
                    colsum = tmp.tile([16, 1], F32, tag="colsum")
                    nc.vector.tensor_copy(colsum[:], ps_cs[:])
                    ps_co = psD.tile([1, NT], F32, tag="small")
                    nc.tensor.matmul(ps_co[:], colsum[:], tri16[:])
                    coloff = tmp.tile([1, NT], F32, tag="coloff")
                    nc.vector.tensor_copy(coloff[:], ps_co[:])
                    ps_pos = psD.tile([P, NT], F32, tag="small")
                    nc.tensor.matmul(ps_pos[:], tri128[:], mask[:],
                                     start=True, stop=False)
                    nc.tensor.matmul(ps_pos[:], ones1x[:], coloff[:],
                                     start=False, stop=True)
                    mm1 = tmp.tile([P, NT], F32, tag="mm1")
                    nc.vector.tensor_scalar_add(mm1[:], mask[:], -1.0)
                    pos = tmp.tile([P, NT], F32, tag="pos")
                    # pos_m = pos*mask + (mask-1):  -1 where unselected
                    nc.vector.tensor_mul(pos[:], ps_pos[:], mask[:])
                    nc.vector.tensor_add(pos[:], pos[:], mm1[:])
                    for t in range(NT):
                        nc.vector.tensor_scalar(
                            O[:, t, :], iotaC[:], pos[:, t:t + 1], None,
                            OP.is_equal,
                        )
                    # gather: dc-pairs, t-inner so the first pair chases O
                    for pr in range(KD // 2):
                        pg0 = psG.tile([P, C], F32, tag="g0", name=f"pg0_{pr}")
                        pg1 = psG.tile([P, C], F32, tag="g1", name=f"pg1_{pr}")
                        d0 = 2 * pr
                        for t in range(NT):
                            nc.tensor.matmul(
                                pg0[:], x_tok[:, t, d0 * P:(d0 + 1) * P],
                                O[:, t, :],
                                start=(t == 0), stop=(t == NT - 1),
                            )
                            nc.tensor.matmul(
                                pg1[:], x_tok[:, t, (d0 + 1) * P:(d0 + 2) * P],
                                O[:, t, :],
                                start=(t == 0), stop=(t == NT - 1),
                            )
                        nc.vector.tensor_copy(xcT[:, d0, :], pg0[:])
                        nc.vector.tensor_copy(xcT[:, d0 + 1, :], pg1[:])

                    # ---------------- aux loss (overlaps gather) ----------------
                    exp_all = tx.tile([P, NT, E], F32, tag="exp_all")
                    nc.scalar.activation(exp_all[:], lt_all[:], AF.Exp)
                    sumexp = tx.tile([P, NT], F32, tag="sumexp")
                    nc.vector.tensor_reduce(
                        sumexp[:], exp_all[:], mybir.AxisListType.X, OP.add
                    )
                    lse = tx.tile([P, NT], F32, tag="lse")
                    lse_sum = tx.tile([P, 1], F32, tag="lse_sum")
                    nc.scalar.activation(
                        lse[:], sumexp[:], AF.Ln, accum_out=lse_sum[:]
                    )
                    ps_z = psD.tile([1, 1], F32, tag="small")
                    nc.tensor.matmul(ps_z[:], lse_sum[:], ones128[:])
                    recip = tx.tile([P, NT], F32, tag="recip")
                    nc.vector.reciprocal(recip[:], sumexp[:])
                    probs = tx.tile([P, NT, E], F32, tag="probs")
                    for t in range(NT):
                        nc.vector.tensor_scalar(
                            probs[:, t, :], exp_all[:, t, :],
                            recip[:, t:t + 1], None, OP.mult,
                        )
                    load = tx.tile([P, NT // 2, E], F32, tag="load")
                    nc.vector.tensor_add(
                        load[:], probs[:, 0:NT // 2, :], probs[:, NT // 2:NT, :]
                    )
                    s1 = tx.tile([P, 1], F32, tag="s1")
                    nc.vector.tensor_reduce(
                        s1[:], load[:], mybir.AxisListType.XY, OP.add
                    )
                    sq = tx.tile([P, NT // 2, E], F32, tag="sq")
                    s2 = tx.tile([P, 1], F32, tag="s2")
                    nc.scalar.activation(sq[:], load[:], AF.Square,
                                         accum_out=s2[:])
                    stats = tx.tile([P, 2], F32, tag="stats")
                    nc.vector.tensor_copy(stats[:, 0:1], s1[:])
                    nc.vector.tensor_copy(stats[:, 1:2], s2[:])
                    ps_s = psD.tile([1, 2], F32, tag="small")
                    nc.tensor.matmul(ps_s[:], ones128[:], stats[:])
                    st = tx.tile([1, 2], F32, tag="st")
                    nc.vector.tensor_copy(st[:], ps_s[:])
                    zt = tx.tile([1, 1], F32, tag="zt")
                    nc.vector.tensor_copy(zt[:], ps_z[:])
                    # scalar math
                    NL = float(1024 * E)
                    m2 = tx.tile([1, 1], F32, tag="m2")
                    nc.vector.tensor_mul(m2[:], st[:, 0:1], st[:, 0:1])
                    t3 = tx.tile([1, 1], F32, tag="t3")
                    nc.vector.tensor_scalar_mul(t3[:], m2[:], 1.0 / NL)
                    v1 = tx.tile([1, 1], F32, tag="v1")
                    nc.vector.tensor_sub(v1[:], st[:, 1:2], t3[:])
                    var = tx.tile([1, 1], F32, tag="var")
                    nc.vector.tensor_scalar_mul(var[:], v1[:], 1.0 / (NL - 1.0))
                    mean2 = tx.tile([1, 1], F32, tag="mean2")
                    nc.vector.tensor_scalar_mul(mean2[:], m2[:], 1.0 / (NL * NL))
                    rm = tx.tile([1, 1], F32, tag="rm")
                    nc.vector.reciprocal(rm[:], mean2[:])
                    ratio = tx.tile([1, 1], F32, tag="ratio")
                    nc.vector.tensor_mul(ratio[:], var[:], rm[:])
                    mz = tx.tile([1, 1], F32, tag="mz")
                    nc.vector.tensor_scalar_mul(mz[:], zt[:], 1.0 / float(N))
                    z2 = tx.tile([1, 1], F32, tag="z2")
                    nc.vector.tensor_mul(z2[:], mz[:], mz[:])
                    t4 = tx.tile([1, 1], F32, tag="t4")
                    nc.vector.tensor_add(t4[:], z2[:], ratio[:])
                    auxv = tx.tile([1, 1], F32, tag="auxv")
                    nc.vector.tensor_scalar_mul(auxv[:], t4[:], 1.0e-3)
                    nc.scalar.dma_start(d_aux[:], auxv[:])
                nc.scalar.dma_start(d_sel[:], sel[:])

                # ---------------- Phase F: FFN ----------------
                with (
                    tc.tile_pool(name="psF", bufs=1, space="PSUM") as psF,
                ):
                    ps_yct = [
                        psF.tile([P, C], F32, tag=f"yt{i}", name=f"ps_yct{i}")
                        for i in range(6)
                    ]
                    for hc in range(KH):
                        w1t = w12.tile([P, KD, P], BF16, tag="w1")
                        nc.sync.dma_start(w1t[:], d_w1[hc][:])
                        w2t = w12.tile([P, KD, P], BF16, tag="w2")
                        nc.sync.dma_start(w2t[:], d_w2[hc][:])
                        nc.sync.dma_start(w3all[:, hc, :], d_w3[hc][:])
                        ps_a = psF.tile([P, C], F32, tag="a")
                        ps_b = psF.tile([P, C], F32, tag="b")
                        for kt in range(KD):
                            nc.tensor.matmul(
                                ps_a[:], w1t[:, kt, :], xcT[:, kt, :],
                                start=(kt == 0), stop=(kt == KD - 1),
                            )
                        for kt in range(KD):
                            nc.tensor.matmul(
                                ps_b[:], w2t[:, kt, :], xcT[:, kt, :],
                                start=(kt == 0), stop=(kt == KD - 1),
                            )
                        sil = silp.tile([P, C], F32, tag="sil")
                        nc.scalar.activation(sil[:], ps_a[:], AF.Silu)
                        nc.vector.tensor_mul(hT[:, hc, :], sil[:], ps_b[:])
                        # FFN3 pass A: d-chunks 0..5, accumulated across hc
                        for i in range(6):
                            nc.tensor.matmul(
                                ps_yct[i][:],
                                w3all[:, hc, i * P:(i + 1) * P],
                                hT[:, hc, :],
                                start=(hc == 0), stop=(hc == KH - 1),
                            )
                    ycte = ycp.tile([P, 6, C], F32, tag="ycteA")
                    for i in range(6):
                        if i % 2 == 0:
                            nc.scalar.copy(ycte[:, i, :], ps_yct[i][:])
                        else:
                            nc.vector.tensor_copy(ycte[:, i, :], ps_yct[i][:])
                        nc.sync.dma_start(d_yct[i][:], ycte[:, i, :])
                    # FFN3 pass B: d-chunks 6..7 over resident w3/hT
                    ps_yct2 = [
                        psF.tile([P, C], F32, tag=f"yt{i}", name=f"ps_yct2_{i}")
                        for i in range(2)
                    ]
                    for hc in range(KH):
                        for i in range(2):
                            nc.tensor.matmul(
                                ps_yct2[i][:],
                                w3all[:, hc, (6 + i) * P:(7 + i) * P],
                                hT[:, hc, :],
                                start=(hc == 0), stop=(hc == KH - 1),
                            )
                    ycte2 = ycp.tile([P, 2, C], F32, tag="ycteB")
                    for i in range(2):
                        if i % 2 == 0:
                            nc.scalar.copy(ycte2[:, i, :], ps_yct2[i][:])
                        else:
                            nc.vector.tensor_copy(ycte2[:, i, :], ps_yct2[i][:])
                        nc.sync.dma_start(d_yct[6 + i][:], ycte2[:, i, :])
    nc.compile()
    return nc, names


def _prep_host(x, Wr, W1, W2, W3):
    flat = np.ascontiguousarray(x.reshape(N, D), dtype=np.float32)
    xT = np.ascontiguousarray(flat.T).reshape(KD, P, N)
    x_tok = np.ascontiguousarray(
        flat.reshape(NT, P, D).transpose(1, 0, 2)
    ).astype(ml_dtypes.bfloat16)
    # wrT[p, kt, e] = Wr[e, kt*128+p]
    wrT = np.ascontiguousarray(
        Wr.T.reshape(KD, P, E).transpose(1, 0, 2), dtype=np.float32
    )
    bf = ml_dtypes.bfloat16
    # w1c[e, hc, p, kt, m] = W1[e, hc*128+m, kt*128+p]
    w1c = W1.reshape(E, KH, P, KD, P).transpose(0, 1, 4, 3, 2).astype(bf)
    w2c = W2.reshape(E, KH, P, KD, P).transpose(0, 1, 4, 3, 2).astype(bf)
    w3s = np.ascontiguousarray(
        W3.transpose(0, 2, 1).reshape(E, KH, P, D)
    ).astype(bf)
    consts = dict(
        tri128=np.triu(np.ones((P, P), np.float32), 1),
        tri16=np.triu(np.ones((16, 16), np.float32), 1),
        ones128=np.ones((P, 1), np.float32),
        ones1x=np.ones((1, P), np.float32),
        iotaC=np.broadcast_to(
            np.arange(C, dtype=np.float32), (P, C)
        ).copy(),
        iotam8=np.broadcast_to(
            (np.arange(E) - 8.0).astype(np.float32), (P, NT, E)
        ).copy(),
        ident=np.eye(P, dtype=np.float32),
    )
    return flat, xT, x_tok, wrT, w1c, w2c, w3s, consts


def kernel(x, Wr, W1, W2, W3):
    x = np.asarray(x, dtype=np.float32)
    Wr = np.asarray(Wr, dtype=np.float32)
    W1 = np.asarray(W1, dtype=np.float32)
    W2 = np.asarray(W2, dtype=np.float32)
    W3 = np.asarray(W3, dtype=np.float32)

    if "nc" not in _CACHE:
        _CACHE["nc"] = _build()
    nc, nm = _CACHE["nc"]

    flat, xT, x_tok, wrT, w1c, w2c, w3s, consts = _prep_host(x, Wr, W1, W2, W3)

    in_maps = []
    for c in range(E):
        ecol = np.full((P, 1), float(c), np.float32)
        in_maps.append({
            nm["xT"]: xT, nm["xtok"]: x_tok, nm["wrT"]: wrT,
            nm["w1"]: w1c[c], nm["w2"]: w2c[c], nm["w3"]: w3s[c],
            nm["ecol"]: ecol,
            nm["tri128"]: consts["tri128"], nm["tri16"]: consts["tri16"],
            nm["ones128"]: consts["ones128"], nm["ones1x"]: consts["ones1x"],
            nm["iotaC"]: consts["iotaC"], nm["iotam8"]: consts["iotam8"],
            nm["ident"]: consts["ident"],
        })
    res = run_bass_kernel_spmd(nc, in_maps, list(range(E)))
    _CACHE["last_res"] = res

    sel_arr = res.results[0][nm["sel"]]          # [128, 16], token n = t*128+p
    sel_flat = sel_arr.flatten(order="F").astype(np.int64)  # [2048]
    _CACHE["sel_flat"] = sel_flat
    out = np.zeros((N, D), dtype=np.float32)
    flat = x.reshape(N, D)
    for c in range(E):
        idx = np.where(sel_flat == c)[0]
        ycT = res.results[c][nm["yc"]].reshape(D, C)  # [KD*P, C]
        take = min(len(idx), C)
        out[idx[:take]] = ycT[:, :take].T
        if len(idx) > C:
            # capacity overflow: exact host fallback for the tail tokens
            xo = flat[idx[C:]]
            a = xo @ W1[c].T
            h = (a / (1.0 + np.exp(-a))) * (xo @ W2[c].T)
            out[idx[C:]] = h @ W3[c].T
    aux = np.float32(res.results[0][nm["aux"]][0, 0])
    return out.reshape(2, 1024, D), aux


if __name__ == "__main__":
    import json
    rng = np.random.default_rng(0)
    x = rng.standard_normal((2, 1024, D), dtype=np.float32)
    Wr = (rng.standard_normal((E, D), dtype=np.float32) / np.sqrt(D))
    W1 = (rng.standard_normal((E, HID, D), dtype=np.float32) / np.sqrt(D))
    W2 = (rng.standard_normal((E, HID, D), dtype=np.float32) / np.sqrt(D))
    W3 = (rng.standard_normal((E, D, HID), dtype=np.float32) / np.sqrt(HID))
    out, aux = kernel(x, Wr, W1, W2, W3)
    print("out", out.shape, out.dtype, "aux", aux)


# revision 26
# speedup vs baseline: 1.0087x; 1.0021x over previous
"""Expert-parallel MoE (top-1 routing) Bass kernel for Trainium2, 8 cores.

Strategy (core c = expert c, SPMD — one program, per-core weight slices):
  1. Router logits via exact fp32 PE matmul (logitsT [E, N], experts on
     partitions, tokens on free dim).
  2. PE-transpose to token-major [128, 16, E]; per-token argmax via
     reduce_max + is_equal + min(eq*(iota-8))+8  (first-max tie-break,
     matching jax.lax.top_k).
  3. mask = (sel == c); compaction positions via matmul-based exclusive
     cumsum (strict-upper-triangular one-matrices); unselected -> -1.
  4. One-hot dispatch matrix O [N, C] bf16 built by f32 iota is_equal;
     gather matmul xcT[d, j] = sum_n x[n, d] * O[n, j]  (bf16), paced so
     the first d-chunk pair chases the O build.
  5. SwiGLU FFN on the C compacted tokens (bf16 matmuls, fp32 PSUM):
     per 128-row H chunk: a = W1 xc, b = W2 xc, h = silu(a)*b; the
     combine ycT[d, c] += W3 h runs as two passes (6 then 2 PSUM
     accumulators) over a resident W3, giving zero M-padding waste.
  6. Aux loss (z-loss + load-balance CV^2) fully on device from the
     token-major logits (|logit| <~ 6 for N(0,1)-scaled inputs, so exp
     without max-subtraction is safe in fp32).
  7. Host unshard: out[token_ids(c)] = ycT_c.T[:count_c]; aux from core 0.
     Tokens beyond the C=320 capacity (never observed; max count 298)
     fall back to an exact host-side computation.

Inputs (full, unsharded): x [2,1024,1024] f32, Wr [8,1024] f32,
W1 [8,3584,1024] f32, W2 [8,3584,1024] f32, W3 [8,1024,3584] f32.
Returns (out [2,1024,1024] f32, aux_loss f32 scalar) like the reference.
"""

import os
# The axon client in this container has no NTFF profiling hook; a stray
# BASS_TRACE=1 in the environment would crash run_bass_kernel_spmd.
os.environ["BASS_NEVER_TRACE"] = "1"

import numpy as np
import ml_dtypes

import concourse.bacc as bacc
import concourse.tile as tile
from concourse.tile import add_dep_helper
import concourse.mybir as mybir
from concourse.bass_utils import run_bass_kernel_spmd

P = 128
D = 1024
HID = 3584
E = 8
N = 2048
C = 320          # per-expert token capacity (max observed count ~298)
KD = D // P      # 8 k-tiles over D
KH = HID // P    # 28 k-tiles over H
NT = N // P      # 16 token tiles
F32 = mybir.dt.float32
F16 = mybir.dt.float16
BF16 = mybir.dt.bfloat16
AF = mybir.ActivationFunctionType
OP = mybir.AluOpType

_CACHE = {}


def _build():
    nc = bacc.Bacc(None, target_bir_lowering=False)
    names = {}
    with tile.TileContext(nc) as tc:
        with tc.tile_pool(name="dram", bufs=1, space="DRAM") as dram:
            # ---------------- I/O ----------------
            d_xT = dram.tile([KD, P, N], F32, kind="ExternalInput")
            d_xtok = dram.tile([P, NT, D], BF16, kind="ExternalInput")
            d_wrT = dram.tile([P, KD, E], F32, kind="ExternalInput")
            d_w1 = dram.tile([KH, P, KD, P], BF16, kind="ExternalInput")
            d_w2 = dram.tile([KH, P, KD, P], BF16, kind="ExternalInput")
            d_w3 = dram.tile([KH, P, D], BF16, kind="ExternalInput")
            d_ecol = dram.tile([P, 1], F32, kind="ExternalInput")
            # constants
            d_tri128 = dram.tile([P, P], F32, kind="ExternalInput")
            d_tri16 = dram.tile([16, 16], F32, kind="ExternalInput")
            d_ones128 = dram.tile([P, 1], F32, kind="ExternalInput")
            d_ones1x = dram.tile([1, P], F32, kind="ExternalInput")
            d_iotaC = dram.tile([P, C], F32, kind="ExternalInput")
            d_iotam8 = dram.tile([P, NT, E], F32, kind="ExternalInput")
            d_ident = dram.tile([P, P], F32, kind="ExternalInput")
            # outputs
            d_yct = dram.tile([KD, P, C], F32, kind="ExternalOutput")
            d_sel = dram.tile([P, NT], F32, kind="ExternalOutput")
            d_aux = dram.tile([1, 1], F32, kind="ExternalOutput")
            names = dict(
                xT=d_xT.name, xtok=d_xtok.name, wrT=d_wrT.name,
                w1=d_w1.name, w2=d_w2.name, w3=d_w3.name, ecol=d_ecol.name,
                tri128=d_tri128.name, tri16=d_tri16.name,
                ones128=d_ones128.name, ones1x=d_ones1x.name,
                iotaC=d_iotaC.name, iotam8=d_iotam8.name, ident=d_ident.name,
                yc=d_yct.name, sel=d_sel.name, aux=d_aux.name,
            )

            with (
                tc.tile_pool(name="const", bufs=1) as constp,
                tc.tile_pool(name="persist", bufs=1) as persist,
                tc.tile_pool(name="w12", bufs=3) as w12,
                tc.tile_pool(name="w3p", bufs=3) as w3p,
                tc.tile_pool(name="silp", bufs=2) as silp,
                tc.tile_pool(name="ycp", bufs=1) as ycp,
            ):
                # persistent tensors (x_tok DMA deferred until gather phase)
                x_tok = persist.tile([P, NT, D], BF16, tag="x_tok")
                wr_all = persist.tile([P, KD, E], F32, tag="wr_all")
                nc.scalar.dma_start(wr_all[:], d_wrT[:])
                logits_sb = persist.tile([E, N], F32, tag="logits_sb")
                lt_all = persist.tile([P, NT, E], F32, tag="lt_all")
                O = persist.tile([P, NT, C], BF16, tag="O")
                xcT = persist.tile([P, KD, C], BF16, tag="xcT")
                hT = persist.tile([P, KH, C], BF16, tag="hT")
                w3all = persist.tile([P, KH, D], BF16, tag="w3all")
                sel = persist.tile([P, NT], F32, tag="sel")
                mask = persist.tile([P, NT], F32, tag="mask")

                # ---------------- Phase R: router (fp32) ----------------
                with (
                    tc.tile_pool(name="xr", bufs=3) as xrp,
                    tc.tile_pool(name="psR", bufs=1, space="PSUM") as psR,
                ):
                    ps_l = [
                        psR.tile([E, 512], F32, tag=f"l{j}", name=f"ps_l{j}")
                        for j in range(4)
                    ]
                    xts = []
                    last_xt_dma = None
                    for kt in range(KD):
                        xt = xrp.tile([P, N], F32, tag="xr", name=f"xt{kt}")
                        for q in range(4):
                            qsl = slice(q * 512, (q + 1) * 512)
                            last_xt_dma = nc.sync.dma_start(
                                xt[:, qsl], d_xT[kt][:, qsl])
                        xts.append(xt)
                    # constants (issued after the router stream in program order)
                    tri128 = constp.tile([P, P], F32, tag="tri128")
                    nc.sync.dma_start(tri128[:], d_tri128[:])
                    tri16 = constp.tile([16, 16], F32, tag="tri16")
                    nc.sync.dma_start(tri16[:], d_tri16[:])
                    ones128 = constp.tile([P, 1], F32, tag="ones128")
                    nc.sync.dma_start(ones128[:], d_ones128[:])
                    ones1x = constp.tile([1, P], F32, tag="ones1x")
                    nc.sync.dma_start(ones1x[:], d_ones1x[:])
                    iotaC = constp.tile([P, C], F32, tag="iotaC")
                    nc.sync.dma_start(iotaC[:], d_iotaC[:])
                    iotam8 = constp.tile([P, NT, E], F32, tag="iotam8")
                    nc.sync.dma_start(iotam8[:], d_iotam8[:])
                    ident = constp.tile([P, P], F32, tag="ident")
                    nc.sync.dma_start(ident[:], d_ident[:])
                    ecol = constp.tile([P, 1], F32, tag="ecol")
                    nc.sync.dma_start(ecol[:], d_ecol[:])
                    for kt in range(KD):
                        for j in range(4):
                            sl = slice(j * 512, (j + 1) * 512)
                            nc.tensor.matmul(
                                ps_l[j][:], wr_all[:, kt, :], xts[kt][:, sl],
                                start=(kt == 0), stop=(kt == KD - 1),
                            )
                    for j in range(4):
                        sl = slice(j * 512, (j + 1) * 512)
                        if j % 2 == 0:
                            nc.scalar.copy(logits_sb[:, sl], ps_l[j][:])
                        else:
                            nc.vector.tensor_copy(logits_sb[:, sl], ps_l[j][:])
                # x_tok transfer queued right after the router stream
                prev_dma = last_xt_dma
                for q in range(4):
                    qsl = slice(q * 4, (q + 1) * 4)
                    xtok_dma = nc.sync.dma_start(x_tok[:, qsl, :], d_xtok[:, qsl, :])
                    add_dep_helper(xtok_dma.ins, prev_dma.ins, sync=False,
                                   reason="x_tok after router stream, in order")
                    prev_dma = xtok_dma

                # ------- Phase T/X/G: argmax, pos, aux, one-hot, gather -------
                with (
                    tc.tile_pool(name="psD", bufs=2, space="PSUM") as psD,
                    tc.tile_pool(name="tmpT", bufs=1) as tmp,
                    tc.tile_pool(name="tmpX", bufs=1) as tx,
                ):
                    m_max = tmp.tile([P, NT], F32, tag="m_max")
                    eq = tmp.tile([P, NT, E], F32, tag="eq")
                    with tc.tile_pool(name="psT", bufs=2, space="PSUM") as psT:
                        for t in range(NT):
                            pst = psT.tile([P, E], F32, tag="pst")
                            nc.tensor.transpose(
                                pst[:], logits_sb[:, t * P:(t + 1) * P],
                                ident[:E, :E]
                            )
                            if t % 2 == 0:
                                nc.vector.tensor_copy(lt_all[:, t, :], pst[:])
                            else:
                                nc.scalar.copy(lt_all[:, t, :], pst[:])
                            nc.vector.tensor_reduce(
                                m_max[:, t:t + 1], lt_all[:, t, :],
                                mybir.AxisListType.X, OP.max
                            )
                            nc.vector.tensor_scalar(
                                eq[:, t, :], lt_all[:, t, :],
                                m_max[:, t:t + 1], None, OP.is_equal,
                            )
                    nc.vector.tensor_mul(eq[:], eq[:], iotam8[:])
                    selm = tmp.tile([P, NT], F32, tag="selm")
                    nc.vector.tensor_reduce(
                        selm[:], eq[:], mybir.AxisListType.X, OP.min
                    )
                    nc.vector.tensor_scalar_add(sel[:], selm[:], 8.0)
                    nc.vector.tensor_scalar(
                        mask[:], sel[:], ecol[:, 0:1], None, OP.is_equal
                    )
                    # exclusive cumsum of mask in token order (n = t*128 + p)
                    ps_cs = psD.tile([16, 1], F32, tag="small")
                    nc.tensor.matmul(ps_cs[:], mask[:], ones128[:])
                    colsum = tmp.tile([16, 1], F32, tag="colsum")
                    nc.vector.tensor_copy(colsum[:], ps_cs[:])
                    ps_co = psD.tile([1, NT], F32, tag="small")
                    nc.tensor.matmul(ps_co[:], colsum[:], tri16[:])
                    coloff = tmp.tile([1, NT], F32, tag="coloff")
                    nc.vector.tensor_copy(coloff[:], ps_co[:])
                    ps_pos = psD.tile([P, NT], F32, tag="small")
                    nc.tensor.matmul(ps_pos[:], tri128[:], mask[:],
                                     start=True, stop=False)
                    nc.tensor.matmul(ps_pos[:], ones1x[:], coloff[:],
                                     start=False, stop=True)
                    mm1 = tmp.tile([P, NT], F32, tag="mm1")
                    nc.vector.tensor_scalar_add(mm1[:], mask[:], -1.0)
                    pos = tmp.tile([P, NT], F32, tag="pos")
                    # pos_m = pos*mask + (mask-1):  -1 where unselected
                    nc.vector.tensor_mul(pos[:], ps_pos[:], mask[:])
                    nc.vector.tensor_add(pos[:], pos[:], mm1[:])
                    for t in range(NT):
                        nc.vector.tensor_scalar(
                            O[:, t, :], iotaC[:], pos[:, t:t + 1], None,
                            OP.is_equal,
                        )
                    # gather: dc-pairs, t-inner so the first pair chases O
                    with tc.tile_pool(name="psG", bufs=3, space="PSUM") as psG:
                        for pr in range(KD // 2):
                            pg0 = psG.tile([P, C], F32, tag="g0",
                                           name=f"pg0_{pr}")
                            pg1 = psG.tile([P, C], F32, tag="g1",
                                           name=f"pg1_{pr}")
                            d0 = 2 * pr
                            for t in range(NT):
                                nc.tensor.matmul(
                                    pg0[:], x_tok[:, t, d0 * P:(d0 + 1) * P],
                                    O[:, t, :],
                                    start=(t == 0), stop=(t == NT - 1),
                                )
                                nc.tensor.matmul(
                                    pg1[:],
                                    x_tok[:, t, (d0 + 1) * P:(d0 + 2) * P],
                                    O[:, t, :],
                                    start=(t == 0), stop=(t == NT - 1),
                                )
                            nc.vector.tensor_copy(xcT[:, d0, :], pg0[:])
                            if pr == KD // 2 - 1:
                                nc.scalar.copy(xcT[:, d0 + 1, :], pg1[:])
                            else:
                                nc.vector.tensor_copy(
                                    xcT[:, d0 + 1, :], pg1[:])

                    # ---------------- aux loss (overlaps gather) ----------------
                    exp_all = tx.tile([P, NT, E], F32, tag="exp_all")
                    nc.scalar.activation(exp_all[:], lt_all[:], AF.Exp)
                    sumexp = tx.tile([P, NT], F32, tag="sumexp")
                    nc.vector.tensor_reduce(
                        sumexp[:], exp_all[:], mybir.AxisListType.X, OP.add
                    )
                    lse = tx.tile([P, NT], F32, tag="lse")
                    lse_sum = tx.tile([P, 1], F32, tag="lse_sum")
                    nc.scalar.activation(
                        lse[:], sumexp[:], AF.Ln, accum_out=lse_sum[:]
                    )
                    ps_z = psD.tile([1, 1], F32, tag="small")
                    nc.tensor.matmul(ps_z[:], lse_sum[:], ones128[:])
                    recip = tx.tile([P, NT], F32, tag="recip")
                    nc.vector.reciprocal(recip[:], sumexp[:])
                    probs = tx.tile([P, NT, E], F32, tag="probs")
                    for t in range(NT):
                        nc.vector.tensor_scalar(
                            probs[:, t, :], exp_all[:, t, :],
                            recip[:, t:t + 1], None, OP.mult,
                        )
                    load = tx.tile([P, NT // 2, E], F32, tag="load")
                    nc.vector.tensor_add(
                        load[:], probs[:, 0:NT // 2, :], probs[:, NT // 2:NT, :]
                    )
                    s1 = tx.tile([P, 1], F32, tag="s1")
                    nc.vector.tensor_reduce(
                        s1[:], load[:], mybir.AxisListType.XY, OP.add
                    )
                    sq = tx.tile([P, NT // 2, E], F32, tag="sq")
                    s2 = tx.tile([P, 1], F32, tag="s2")
                    nc.scalar.activation(sq[:], load[:], AF.Square,
                                         accum_out=s2[:])
                    stats = tx.tile([P, 2], F32, tag="stats")
                    nc.vector.tensor_copy(stats[:, 0:1], s1[:])
                    nc.vector.tensor_copy(stats[:, 1:2], s2[:])
                    ps_s = psD.tile([1, 2], F32, tag="small")
                    nc.tensor.matmul(ps_s[:], ones128[:], stats[:])
                    st = tx.tile([1, 2], F32, tag="st")
                    nc.vector.tensor_copy(st[:], ps_s[:])
                    zt = tx.tile([1, 1], F32, tag="zt")
                    nc.vector.tensor_copy(zt[:], ps_z[:])
                    # scalar math
                    NL = float(1024 * E)
                    m2 = tx.tile([1, 1], F32, tag="m2")
                    nc.vector.tensor_mul(m2[:], st[:, 0:1], st[:, 0:1])
                    t3 = tx.tile([1, 1], F32, tag="t3")
                    nc.vector.tensor_scalar_mul(t3[:], m2[:], 1.0 / NL)
                    v1 = tx.tile([1, 1], F32, tag="v1")
                    nc.vector.tensor_sub(v1[:], st[:, 1:2], t3[:])
                    var = tx.tile([1, 1], F32, tag="var")
                    nc.vector.tensor_scalar_mul(var[:], v1[:], 1.0 / (NL - 1.0))
                    mean2 = tx.tile([1, 1], F32, tag="mean2")
                    nc.vector.tensor_scalar_mul(mean2[:], m2[:], 1.0 / (NL * NL))
                    rm = tx.tile([1, 1], F32, tag="rm")
                    nc.vector.reciprocal(rm[:], mean2[:])
                    ratio = tx.tile([1, 1], F32, tag="ratio")
                    nc.vector.tensor_mul(ratio[:], var[:], rm[:])
                    mz = tx.tile([1, 1], F32, tag="mz")
                    nc.vector.tensor_scalar_mul(mz[:], zt[:], 1.0 / float(N))
                    z2 = tx.tile([1, 1], F32, tag="z2")
                    nc.vector.tensor_mul(z2[:], mz[:], mz[:])
                    t4 = tx.tile([1, 1], F32, tag="t4")
                    nc.vector.tensor_add(t4[:], z2[:], ratio[:])
                    auxv = tx.tile([1, 1], F32, tag="auxv")
                    nc.vector.tensor_scalar_mul(auxv[:], t4[:], 1.0e-3)
                    nc.scalar.dma_start(d_aux[:], auxv[:])
                nc.scalar.dma_start(d_sel[:], sel[:])

                # ---------------- Phase F: FFN ----------------
                with (
                    tc.tile_pool(name="psF", bufs=1, space="PSUM") as psF,
                ):
                    ps_yct = [
                        psF.tile([P, C], F32, tag=f"yt{i}", name=f"ps_yct{i}")
                        for i in range(6)
                    ]
                    for hc in range(KH):
                        w1t = w12.tile([P, KD, P], BF16, tag="w1")
                        nc.sync.dma_start(w1t[:], d_w1[hc][:])
                        w2t = w12.tile([P, KD, P], BF16, tag="w2")
                        nc.sync.dma_start(w2t[:], d_w2[hc][:])
                        nc.sync.dma_start(w3all[:, hc, :], d_w3[hc][:])
                        ps_a = psF.tile([P, C], F32, tag="a")
                        ps_b = psF.tile([P, C], F32, tag="b")
                        for kt in range(KD):
                            nc.tensor.matmul(
                                ps_a[:], w1t[:, kt, :], xcT[:, kt, :],
                                start=(kt == 0), stop=(kt == KD - 1),
                            )
                        for kt in range(KD):
                            nc.tensor.matmul(
                                ps_b[:], w2t[:, kt, :], xcT[:, kt, :],
                                start=(kt == 0), stop=(kt == KD - 1),
                            )
                        sil = silp.tile([P, C], F32, tag="sil")
                        nc.scalar.activation(sil[:], ps_a[:], AF.Silu)
                        nc.vector.tensor_mul(hT[:, hc, :], sil[:], ps_b[:])
                        # FFN3 pass A: d-chunks 0..5, accumulated across hc
                        for i in range(6):
                            nc.tensor.matmul(
                                ps_yct[i][:],
                                w3all[:, hc, i * P:(i + 1) * P],
                                hT[:, hc, :],
                                start=(hc == 0), stop=(hc == KH - 1),
                            )
                    ycte = ycp.tile([P, 6, C], F32, tag="ycteA")
                    for i in range(6):
                        if i % 2 == 0:
                            nc.scalar.copy(ycte[:, i, :], ps_yct[i][:])
                        else:
                            nc.vector.tensor_copy(ycte[:, i, :], ps_yct[i][:])
                        nc.sync.dma_start(d_yct[i][:], ycte[:, i, :])
                    # FFN3 pass B: d-chunks 6..7 over resident w3/hT
                    ps_yct2 = [
                        psF.tile([P, C], F32, tag=f"yt{i}", name=f"ps_yct2_{i}")
                        for i in range(2)
                    ]
                    for hc in range(KH):
                        for i in range(2):
                            nc.tensor.matmul(
                                ps_yct2[i][:],
                                w3all[:, hc, (6 + i) * P:(7 + i) * P],
                                hT[:, hc, :],
                                start=(hc == 0), stop=(hc == KH - 1),
                            )
                    ycte2 = ycp.tile([P, 2, C], F32, tag="ycteB")
                    for i in range(2):
                        if i % 2 == 0:
                            nc.scalar.copy(ycte2[:, i, :], ps_yct2[i][:])
                        else:
                            nc.vector.tensor_copy(ycte2[:, i, :], ps_yct2[i][:])
                        nc.sync.dma_start(d_yct[6 + i][:], ycte2[:, i, :])
    nc.compile()
    return nc, names


def _prep_host(x, Wr, W1, W2, W3):
    flat = np.ascontiguousarray(x.reshape(N, D), dtype=np.float32)
    xT = np.ascontiguousarray(flat.T).reshape(KD, P, N)
    x_tok = np.ascontiguousarray(
        flat.reshape(NT, P, D).transpose(1, 0, 2)
    ).astype(ml_dtypes.bfloat16)
    # wrT[p, kt, e] = Wr[e, kt*128+p]
    wrT = np.ascontiguousarray(
        Wr.T.reshape(KD, P, E).transpose(1, 0, 2), dtype=np.float32
    )
    bf = ml_dtypes.bfloat16
    # w1c[e, hc, p, kt, m] = W1[e, hc*128+m, kt*128+p]
    w1c = W1.reshape(E, KH, P, KD, P).transpose(0, 1, 4, 3, 2).astype(bf)
    w2c = W2.reshape(E, KH, P, KD, P).transpose(0, 1, 4, 3, 2).astype(bf)
    w3s = np.ascontiguousarray(
        W3.transpose(0, 2, 1).reshape(E, KH, P, D)
    ).astype(bf)
    consts = dict(
        tri128=np.triu(np.ones((P, P), np.float32), 1),
        tri16=np.triu(np.ones((16, 16), np.float32), 1),
        ones128=np.ones((P, 1), np.float32),
        ones1x=np.ones((1, P), np.float32),
        iotaC=np.broadcast_to(
            np.arange(C, dtype=np.float32), (P, C)
        ).copy(),
        iotam8=np.broadcast_to(
            (np.arange(E) - 8.0).astype(np.float32), (P, NT, E)
        ).copy(),
        ident=np.eye(P, dtype=np.float32),
    )
    return flat, xT, x_tok, wrT, w1c, w2c, w3s, consts


def kernel(x, Wr, W1, W2, W3):
    x = np.asarray(x, dtype=np.float32)
    Wr = np.asarray(Wr, dtype=np.float32)
    W1 = np.asarray(W1, dtype=np.float32)
    W2 = np.asarray(W2, dtype=np.float32)
    W3 = np.asarray(W3, dtype=np.float32)

    if "nc" not in _CACHE:
        _CACHE["nc"] = _build()
    nc, nm = _CACHE["nc"]

    flat, xT, x_tok, wrT, w1c, w2c, w3s, consts = _prep_host(x, Wr, W1, W2, W3)

    in_maps = []
    for c in range(E):
        ecol = np.full((P, 1), float(c), np.float32)
        in_maps.append({
            nm["xT"]: xT, nm["xtok"]: x_tok, nm["wrT"]: wrT,
            nm["w1"]: w1c[c], nm["w2"]: w2c[c], nm["w3"]: w3s[c],
            nm["ecol"]: ecol,
            nm["tri128"]: consts["tri128"], nm["tri16"]: consts["tri16"],
            nm["ones128"]: consts["ones128"], nm["ones1x"]: consts["ones1x"],
            nm["iotaC"]: consts["iotaC"], nm["iotam8"]: consts["iotam8"],
            nm["ident"]: consts["ident"],
        })
    res = run_bass_kernel_spmd(nc, in_maps, list(range(E)))
    _CACHE["last_res"] = res

    sel_arr = res.results[0][nm["sel"]]          # [128, 16], token n = t*128+p
    sel_flat = sel_arr.flatten(order="F").astype(np.int64)  # [2048]
    _CACHE["sel_flat"] = sel_flat
    out = np.zeros((N, D), dtype=np.float32)
    flat = x.reshape(N, D)
    for c in range(E):
        idx = np.where(sel_flat == c)[0]
        ycT = res.results[c][nm["yc"]].reshape(D, C)  # [KD*P, C]
        take = min(len(idx), C)
        out[idx[:take]] = ycT[:, :take].T
        if len(idx) > C:
            # capacity overflow: exact host fallback for the tail tokens
            xo = flat[idx[C:]]
            a = xo @ W1[c].T
            h = (a / (1.0 + np.exp(-a))) * (xo @ W2[c].T)
            out[idx[C:]] = h @ W3[c].T
    aux = np.float32(res.results[0][nm["aux"]][0, 0])
    return out.reshape(2, 1024, D), aux


if __name__ == "__main__":
    import json
    rng = np.random.default_rng(0)
    x = rng.standard_normal((2, 1024, D), dtype=np.float32)
    Wr = (rng.standard_normal((E, D), dtype=np.float32) / np.sqrt(D))
    W1 = (rng.standard_normal((E, HID, D), dtype=np.float32) / np.sqrt(D))
    W2 = (rng.standard_normal((E, HID, D), dtype=np.float32) / np.sqrt(D))
    W3 = (rng.standard_normal((E, D, HID), dtype=np.float32) / np.sqrt(HID))
    out, aux = kernel(x, Wr, W1, W2, W3)
    print("out", out.shape, out.dtype, "aux", aux)
